# revision 12
# baseline (speedup 1.0000x reference)
"""Binarized linear kernel for Trainium2 (8 NeuronCores, SPMD).

Computes out = x @ sign(weight).T with
  x:      [8192, 4096] f32
  weight: [4096, 4096] f32
  out:    [8192, 4096] f32

Strategy (data-parallel over M; mixed-precision contraction):
  - sign(weight) is exactly representable in fp8-e4m3 and bf16, so the
    weight binarization happens on the host for free and the device just
    streams pre-signed weights.
  - The PE's fp8 DoubleRow mode contracts 256 rows per 512-cycle matmul
    (2x the bf16 rate). Quantizing all of x to e4m3 costs 2.66e-2 rel
    error (over the 2e-2 budget), so only NFP8 of the 32 k-chunks of
    128 use the fp8 path (x in e4m3) and the rest stay bf16: total rel
    err ~= 2.66e-2 * sqrt(NFP8/32), while PE time per output tile drops
    from 32 to NFP8/2 (DoubleRow pairs) + (32-NFP8) matmuls.
  - Each core keeps its x shard resident in SBUF (fp8 + bf16 copies,
    K-major so the contraction dim is on partitions) and streams the
    shared pre-signed weights once.
  - DMA issue is the ramp bottleneck (each DMA instruction costs ~0.6us
    on its issuing engine), so loads are batched into few multi-chunk
    3D DMAs split across the two hardware-DGE queues (Sync carries w,
    Scalar carries x) while output stores ride Sync in steady state.
  - Ramp: 20 dummy warm-up matmuls un-throttle the HAM clock gate while
    the first DMAs land; the first n-tile runs k-outer over mj-groups
    of 4 (half-m x loads, half-width w loads) so matmuls chase the DMA
    arrival order; the very last psum bank is drained as two half-width
    accumulation groups to shrink the tail.
"""

import os
import sys

import numpy as np

# Toolchain locations (normally already on sys.path via PYTHONPATH; be
# robust when invoked from a fresh directory/environment).
for _p in (
    "/root/.axon_site",
    "/root/.axon_site/_ro/trn_rl_repo",
    "/root/.axon_site/_ro/pypackages",
    "/opt/trn_rl_repo",
):
    if os.path.isdir(_p) and _p not in sys.path:
        sys.path.append(_p)

import ml_dtypes  # noqa: E402

BF16 = ml_dtypes.bfloat16
FP8 = ml_dtypes.float8_e4m3  # IEEE-style e4m3: matches TRN FP8_EXP4 for |v|<=240

M, K, N = 8192, 4096, 4096
N_CORES = 8
P = 128
N_TILE = 512

NFP8 = 14  # k-chunks (of 128) contracted in fp8 DoubleRow; must be even
NBF = K // P - NFP8  # k-chunks contracted in bf16
KF8 = NFP8 * P  # leading k rows in fp8


def build_nc(mc: int = M // N_CORES, n: int = N):
    """Per-core Bass program (SPMD: same program, different data)."""
    from concourse import bacc, mybir, tile

    DR = mybir.MatmulPerfMode.DoubleRow
    npair = NFP8 // 2
    mj_cnt = mc // P
    np_cnt = n // (2 * N_TILE)  # n-tile PAIRS of 1024 columns

    nc = bacc.Bacc("TRN2", target_bir_lowering=False)

    x8T = nc.dram_tensor("x8T", [KF8, mc], mybir.dt.float8e4, kind="ExternalInput")
    xbT = nc.dram_tensor("xbT", [K - KF8, mc], mybir.dt.bfloat16, kind="ExternalInput")
    w8T = nc.dram_tensor("w8T", [KF8, n], mybir.dt.float8e4, kind="ExternalInput")
    wbT = nc.dram_tensor("wbT", [K - KF8, n], mybir.dt.bfloat16, kind="ExternalInput")
    out = nc.dram_tensor("out", [mc, n], mybir.dt.float32, kind="ExternalOutput")

    x8_ap = x8T[:].rearrange("(ko p) m -> p ko m", p=P)
    xb_ap = xbT[:].rearrange("(ko p) m -> p ko m", p=P)
    w8_ap = w8T[:].rearrange("(ko p) n -> p ko n", p=P)
    wb_ap = wbT[:].rearrange("(ko p) n -> p ko n", p=P)
    out_ap = out[:].rearrange("(t p) n -> t p n", p=P)

    H = N_TILE  # 512

    with tile.TileContext(nc) as tc:
        with (
            tc.tile_pool(name="xres", bufs=1) as xpool,
            tc.tile_pool(name="warmp", bufs=1) as warmpool,
            tc.tile_pool(name="w", bufs=2) as wpool,
            tc.tile_pool(name="o", bufs=6) as opool,
            tc.tile_pool(name="ps", bufs=8, space="PSUM") as pspool,
        ):
            # HAM warm-up: dummy matmuls on a zeroed tile fill the dead
            # PE time while the prologue + first DMAs run, so the real
            # matmul stream starts at 2.4 GHz instead of 1.2.
            warm = warmpool.tile([P, N_TILE], mybir.dt.bfloat16)
            nc.vector.memset(warm[:], 0)
            warm_ps = pspool.tile([P, N_TILE], mybir.dt.float32, tag="ps")
            for _ in range(4):
                nc.tensor.matmul(warm_ps[:], warm[:, :P], warm[:], start=True, stop=True)

            x8_res = xpool.tile([P, NFP8, mc], mybir.dt.float8e4)
            xb_res = xpool.tile([P, NBF, mc], mybir.dt.bfloat16)

            def mm_steps(ps_list, w8_t, wb_t, mjs, nsl_w_list, psl=None):
                """Accumulate 25 chunk-steps into the psum tiles: one MM
                per (mj, n-slice) per step. The group opens with a bf16
                step: its 128-col LDWEIGHTS exposes ~107ns at the tile
                boundary instead of a DoubleRow load's ~213ns."""
                def bf_step(ko, start, stop):
                    for i, mj in enumerate(mjs):
                        msl = slice(mj * P, (mj + 1) * P)
                        for t, ps in enumerate(ps_list[i]):
                            nc.tensor.matmul(
                                ps if psl is None else ps[:, psl],
                                xb_res[:, ko, msl],
                                wb_t[:, ko, nsl_w_list[t]],
                                start=start,
                                stop=stop,
                            )

                bf_step(0, True, False)
                for j in range(npair):
                    for i, mj in enumerate(mjs):
                        msl = slice(mj * P, (mj + 1) * P)
                        for t, ps in enumerate(ps_list[i]):
                            nc.tensor.matmul(
                                ps if psl is None else ps[:, psl],
                                x8_res[:, 2 * j : 2 * j + 2, msl],
                                w8_t[:, 2 * j : 2 * j + 2, nsl_w_list[t]],
                                start=False,
                                stop=False,
                                perf_mode=DR,
                            )
                for ko in range(1, NBF):
                    bf_step(ko, False, ko == NBF - 1)

            def evict(ps_ap, mj, n0, width=N_TILE):
                # Stores ride the GpSimd SWDGE queue so they never
                # head-of-line-block the w prefetches on Sync.
                o_t = opool.tile([P, N_TILE], mybir.dt.float32, name="o_t")
                nc.vector.tensor_copy(out=o_t[:, :width], in_=ps_ap)
                nc.gpsimd.dma_start(out_ap[mj, :, n0 : n0 + width], o_t[:, :width])

            # ---- n-pairs 0..3. Pair 0 is the ramp: its loads are split
            # into halves across the two hardware-DGE queues (x on Scalar,
            # w on Sync) in consumption order, and its compute runs nt0
            # k-outer (two mj-groups of 4) then nt1 mj-outer, chasing the
            # DMA arrivals. Pairs 1..3 use batched w loads (one DMA per
            # dtype, prefetched a full pair ahead) and paired mj-outer.
            for g in range(np_cnt):
                w8_t = wpool.tile([P, NFP8, 2 * N_TILE], mybir.dt.float8e4, name="w8_t")
                wb_t = wpool.tile([P, NBF, 2 * N_TILE], mybir.dt.bfloat16, name="wb_t")
                n0 = g * 2 * N_TILE
                nsl = slice(n0, n0 + 2 * N_TILE)
                if g == 0:
                    # Ramp loads: sub-batched DMAs in consumption order,
                    # x on Scalar / w on Sync (the two hardware-DGE
                    # queues; each queue is a FIFO pipe drawing from one
                    # shared ~370GB/s pool, so per-queue order must match
                    # the matmul consumption schedule).
                    # First two fp8 pairs are per-pair DMAs so the very
                    # first matmuls start ~9us instead of waiting for a
                    # whole block (fine granularity costs ~1.4us of queue
                    # time per pair, so the rest ride batches).
                    third = NBF // 3
                    for j in range(2):
                        ksl = slice(2 * j, 2 * j + 2)
                        nc.scalar.dma_start(x8_res[:, ksl, :H], x8_ap[:, ksl, :H])
                        nc.sync.dma_start(w8_t[:, ksl, :H], w8_ap[:, ksl, :H])
                    ksl = slice(4, NFP8)
                    nc.scalar.dma_start(x8_res[:, ksl, :H], x8_ap[:, ksl, :H])
                    nc.sync.dma_start(w8_t[:, ksl, :H], w8_ap[:, ksl, :H])
                    for b in range(3):
                        ksl = slice(b * third, (b + 1) * third)
                        nc.scalar.dma_start(xb_res[:, ksl, :H], xb_ap[:, ksl, :H])
                        nc.sync.dma_start(wb_t[:, ksl, :H], wb_ap[:, ksl, :H])
                    nc.scalar.dma_start(x8_res[:, :, H:], x8_ap[:, :, H:])
                    nc.sync.dma_start(w8_t[:, :, H:], w8_ap[:, :, H : 2 * H])
                    for b in range(3):
                        ksl = slice(b * third, (b + 1) * third)
                        nc.scalar.dma_start(xb_res[:, ksl, H:], xb_ap[:, ksl, H:])
                        nc.sync.dma_start(wb_t[:, ksl, H:], wb_ap[:, ksl, H : 2 * H])

                    # nt0 (cols 0:512): k-outer over two mj-groups of 4
                    for grp in range(2):
                        pss = [
                            pspool.tile(
                                [P, N_TILE],
                                mybir.dt.float32,
                                name=f"ps0_{grp}_{i}",
                                tag="ps",
                            )
                            for i in range(4)
                        ]
                        mjs = [grp * 4 + i for i in range(4)]
                        mm_steps(
                            [[pss[i][:]] for i in range(4)],
                            w8_t,
                            wb_t,
                            mjs,
                            [slice(0, N_TILE)],
                        )
                        for i in range(4):
                            evict(pss[i][:], mjs[i], 0)

                    # nt1 (cols 512:1024): mj-outer, x resident
                    for mj in range(mj_cnt):
                        ps1 = pspool.tile(
                            [P, N_TILE], mybir.dt.float32, name=f"ps1_{mj}", tag="ps"
                        )
                        mm_steps(
                            [[ps1[:]]], w8_t, wb_t, [mj], [slice(N_TILE, 2 * N_TILE)]
                        )
                        evict(ps1[:], mj, N_TILE)
                    continue
                nc.sync.dma_start(w8_t[:, :, :], w8_ap[:, :, nsl])
                nc.sync.dma_start(wb_t[:, :, :], wb_ap[:, :, nsl])
                for mj in range(mj_cnt):
                    last = g == np_cnt - 1 and mj == mj_cnt - 1
                    ps_a = pspool.tile(
                        [P, N_TILE], mybir.dt.float32, name=f"psa_{g}_{mj}", tag="ps"
                    )
                    if not last:
                        ps_b = pspool.tile(
                            [P, N_TILE], mybir.dt.float32, name=f"psb_{g}_{mj}", tag="ps"
                        )
                        mm_steps(
                            [[ps_a[:], ps_b[:]]],
                            w8_t,
                            wb_t,
                            [mj],
                            [slice(0, N_TILE), slice(N_TILE, 2 * N_TILE)],
                        )
                        evict(ps_a[:], mj, n0)
                        evict(ps_b[:], mj, n0 + N_TILE)
                    else:
                        # Kernel-tail drain: nt0 normally, then nt1 as two
                        # sequential half-width groups so the first half's
                        # copy + store complete under the second half's
                        # matmuls.
                        mm_steps([[ps_a[:]]], w8_t, wb_t, [mj], [slice(0, N_TILE)])
                        evict(ps_a[:], mj, n0)
                        ps_b = pspool.tile(
                            [P, N_TILE], mybir.dt.float32, name=f"psbl_{g}_{mj}", tag="ps"
                        )
                        o_t = opool.tile([P, N_TILE], mybir.dt.float32)
                        h = N_TILE // 2
                        for half in range(2):
                            hsl = slice(N_TILE + half * h, N_TILE + (half + 1) * h)
                            psl = slice(half * h, (half + 1) * h)
                            mm_steps(
                                [[ps_b]], w8_t, wb_t, [mj], [hsl], psl=psl
                            )
                            nc.vector.tensor_copy(out=o_t[:, psl], in_=ps_b[:, psl])
                            nc.gpsimd.dma_start(
                                out_ap[
                                    mj, :, n0 + N_TILE + half * h : n0 + N_TILE + (half + 1) * h
                                ],
                                o_t[:, psl],
                            )

    return nc


_CACHE: dict = {}


def _get_finalized_nc():
    nc = _CACHE.get("nc")
    if nc is None:
        nc = build_nc()
        nc.finalize()
        _CACHE["nc"] = nc
    return nc


def _host_prep(x: np.ndarray, weight: np.ndarray):
    """Pre-sign weights, split-cast x, K-major transposes.

    Returns (x8_global [8*KF8, mc] fp8, xb_global [8*(K-KF8), mc] bf16,
             w8T [KF8, N] fp8, wbT [K-KF8, N] bf16)."""
    mc = M // N_CORES
    wb = np.sign(weight).astype(np.float32, copy=False)
    wT8 = np.ascontiguousarray(wb[:, :KF8].astype(FP8).view(np.uint8).T).view(FP8)
    wTb = (
        np.ascontiguousarray(wb[:, KF8:].astype(BF16).view(np.uint16).T).view(BF16)
    )
    # x: per-core K-major shards (transpose via integer views: ml_dtypes
    # object paths are slow for strided copies).
    x8 = np.ascontiguousarray(
        x[:, :KF8].astype(FP8).view(np.uint8).reshape(N_CORES, mc, KF8).transpose(0, 2, 1)
    )
    x8_global = x8.reshape(N_CORES * KF8, mc).view(FP8)
    xb = np.ascontiguousarray(
        x[:, KF8:].astype(BF16).view(np.uint16).reshape(N_CORES, mc, K - KF8).transpose(0, 2, 1)
    )
    xb_global = xb.reshape(N_CORES * (K - KF8), mc).view(BF16)
    return x8_global, xb_global, wT8, wTb


def make_in_maps(x: np.ndarray, weight: np.ndarray):
    x8_global, xb_global, wT8, wTb = _host_prep(x, weight)
    kb = K - KF8
    return [
        {
            "x8T": x8_global[c * KF8 : (c + 1) * KF8],
            "xbT": xb_global[c * kb : (c + 1) * kb],
            "w8T": wT8,
            "wbT": wTb,
        }
        for c in range(N_CORES)
    ]


def kernel(x: np.ndarray, weight: np.ndarray) -> np.ndarray:
    x = np.asarray(x)
    weight = np.asarray(weight)
    assert x.shape == (M, K) and weight.shape == (N, K)

    nc = _get_finalized_nc()
    from concourse.bass_utils import run_bass_kernel_spmd

    in_maps = make_in_maps(x, weight)
    try:
        res = run_bass_kernel_spmd(nc, in_maps, core_ids=list(range(N_CORES)))
    except Exception:
        # Transient device hiccups (e.g. NRT_EXEC_UNIT_UNRECOVERABLE) have
        # been observed once across many runs; one retry clears them.
        res = run_bass_kernel_spmd(nc, in_maps, core_ids=list(range(N_CORES)))
    out = np.concatenate([res.results[c]["out"] for c in range(N_CORES)], axis=0)
    return np.ascontiguousarray(out.astype(np.float32, copy=False))


# revision 13
# speedup vs baseline: 1.0125x; 1.0125x over previous
"""Binarized linear kernel for Trainium2 (8 NeuronCores, SPMD).

Computes out = x @ sign(weight).T with
  x:      [8192, 4096] f32
  weight: [4096, 4096] f32
  out:    [8192, 4096] f32

Strategy (data-parallel over M; mixed-precision contraction):
  - sign(weight) is exactly representable in fp8-e4m3 and bf16, so the
    weight binarization happens on the host for free and the device just
    streams pre-signed weights.
  - The PE's fp8 DoubleRow mode contracts 256 rows per 512-cycle matmul
    (2x the bf16 rate). Quantizing all of x to e4m3 costs 2.66e-2 rel
    error (over the 2e-2 budget), so only NFP8 of the 32 k-chunks of
    128 use the fp8 path (x in e4m3) and the rest stay bf16: total rel
    err ~= 2.66e-2 * sqrt(NFP8/32), while PE time per output tile drops
    from 32 to NFP8/2 (DoubleRow pairs) + (32-NFP8) matmuls.
  - Each core keeps its x shard resident in SBUF (fp8 + bf16 copies,
    K-major so the contraction dim is on partitions) and streams the
    shared pre-signed weights once.
  - DMA issue is the ramp bottleneck (each DMA instruction costs ~0.6us
    on its issuing engine), so loads are batched into few multi-chunk
    3D DMAs split across the two hardware-DGE queues (Sync carries w,
    Scalar carries x) while output stores ride Sync in steady state.
  - Ramp: 20 dummy warm-up matmuls un-throttle the HAM clock gate while
    the first DMAs land; the first n-tile runs k-outer over mj-groups
    of 4 (half-m x loads, half-width w loads) so matmuls chase the DMA
    arrival order; the very last psum bank is drained as two half-width
    accumulation groups to shrink the tail.
"""

import os
import sys

import numpy as np

# Toolchain locations (normally already on sys.path via PYTHONPATH; be
# robust when invoked from a fresh directory/environment).
for _p in (
    "/root/.axon_site",
    "/root/.axon_site/_ro/trn_rl_repo",
    "/root/.axon_site/_ro/pypackages",
    "/opt/trn_rl_repo",
):
    if os.path.isdir(_p) and _p not in sys.path:
        sys.path.append(_p)

import ml_dtypes  # noqa: E402

BF16 = ml_dtypes.bfloat16
FP8 = ml_dtypes.float8_e4m3  # IEEE-style e4m3: matches TRN FP8_EXP4 for |v|<=240

M, K, N = 8192, 4096, 4096
N_CORES = 8
P = 128
N_TILE = 512

NFP8 = 14  # k-chunks (of 128) contracted in fp8 DoubleRow; must be even
NBF = K // P - NFP8  # k-chunks contracted in bf16
KF8 = NFP8 * P  # leading k rows in fp8


def build_nc(mc: int = M // N_CORES, n: int = N):
    """Per-core Bass program (SPMD: same program, different data)."""
    from concourse import bacc, mybir, tile

    DR = mybir.MatmulPerfMode.DoubleRow
    npair = NFP8 // 2
    mj_cnt = mc // P
    np_cnt = n // (2 * N_TILE)  # n-tile PAIRS of 1024 columns

    nc = bacc.Bacc("TRN2", target_bir_lowering=False)

    x8T = nc.dram_tensor("x8T", [KF8, mc], mybir.dt.float8e4, kind="ExternalInput")
    xbT = nc.dram_tensor("xbT", [K - KF8, mc], mybir.dt.bfloat16, kind="ExternalInput")
    w8T = nc.dram_tensor("w8T", [KF8, n], mybir.dt.float8e4, kind="ExternalInput")
    wbT = nc.dram_tensor("wbT", [K - KF8, n], mybir.dt.bfloat16, kind="ExternalInput")
    out = nc.dram_tensor("out", [mc, n], mybir.dt.float32, kind="ExternalOutput")

    x8_ap = x8T[:].rearrange("(ko p) m -> p ko m", p=P)
    xb_ap = xbT[:].rearrange("(ko p) m -> p ko m", p=P)
    w8_ap = w8T[:].rearrange("(ko p) n -> p ko n", p=P)
    wb_ap = wbT[:].rearrange("(ko p) n -> p ko n", p=P)
    out_ap = out[:].rearrange("(t p) n -> t p n", p=P)

    H = N_TILE  # 512

    with tile.TileContext(nc) as tc:
        with (
            tc.tile_pool(name="xres", bufs=1) as xpool,
            tc.tile_pool(name="warmp", bufs=1) as warmpool,
            tc.tile_pool(name="w", bufs=2) as wpool,
            tc.tile_pool(name="o", bufs=6) as opool,
            tc.tile_pool(name="ps", bufs=8, space="PSUM") as pspool,
        ):
            # HAM warm-up: dummy matmuls on a zeroed tile fill the dead
            # PE time while the prologue + first DMAs run, so the real
            # matmul stream starts at 2.4 GHz instead of 1.2.
            warm = warmpool.tile([P, N_TILE], mybir.dt.bfloat16)
            nc.vector.memset(warm[:], 0)
            warm_ps = pspool.tile([P, N_TILE], mybir.dt.float32, tag="ps")
            for _ in range(4):
                nc.tensor.matmul(warm_ps[:], warm[:, :P], warm[:], start=True, stop=True)

            x8_res = xpool.tile([P, NFP8, mc], mybir.dt.float8e4)
            xb_res = xpool.tile([P, NBF, mc], mybir.dt.bfloat16)

            def mm_steps(ps_list, w8_t, wb_t, mjs, nsl_w_list, psl=None):
                """Accumulate 25 chunk-steps into the psum tiles: one MM
                per (mj, n-slice) per step, fp8 DoubleRow pairs first
                (matching the ramp's DMA arrival order)."""
                for j in range(npair):
                    for i, mj in enumerate(mjs):
                        msl = slice(mj * P, (mj + 1) * P)
                        for t, ps in enumerate(ps_list[i]):
                            nc.tensor.matmul(
                                ps if psl is None else ps[:, psl],
                                x8_res[:, 2 * j : 2 * j + 2, msl],
                                w8_t[:, 2 * j : 2 * j + 2, nsl_w_list[t]],
                                start=(j == 0),
                                stop=False,
                                perf_mode=DR,
                            )
                for ko in range(NBF):
                    for i, mj in enumerate(mjs):
                        msl = slice(mj * P, (mj + 1) * P)
                        for t, ps in enumerate(ps_list[i]):
                            nc.tensor.matmul(
                                ps if psl is None else ps[:, psl],
                                xb_res[:, ko, msl],
                                wb_t[:, ko, nsl_w_list[t]],
                                start=False,
                                stop=(ko == NBF - 1),
                            )

            def evict(ps_ap, mj, n0, width=N_TILE):
                # Stores ride the GpSimd SWDGE queue so they never
                # head-of-line-block the w prefetches on Sync.
                o_t = opool.tile([P, N_TILE], mybir.dt.float32, name="o_t")
                nc.vector.tensor_copy(out=o_t[:, :width], in_=ps_ap)
                nc.gpsimd.dma_start(out_ap[mj, :, n0 : n0 + width], o_t[:, :width])

            # ---- n-pairs 0..3. Pair 0 is the ramp: its loads are split
            # into halves across the two hardware-DGE queues (x on Scalar,
            # w on Sync) in consumption order, and its compute runs nt0
            # k-outer (two mj-groups of 4) then nt1 mj-outer, chasing the
            # DMA arrivals. Pairs 1..3 use batched w loads (one DMA per
            # dtype, prefetched a full pair ahead) and paired mj-outer.
            for g in range(np_cnt):
                w8_t = wpool.tile([P, NFP8, 2 * N_TILE], mybir.dt.float8e4, name="w8_t")
                wb_t = wpool.tile([P, NBF, 2 * N_TILE], mybir.dt.bfloat16, name="wb_t")
                n0 = g * 2 * N_TILE
                nsl = slice(n0, n0 + 2 * N_TILE)
                if g == 0:
                    # Ramp loads: sub-batched DMAs in consumption order,
                    # x on Scalar / w on Sync (the two hardware-DGE
                    # queues; each queue is a FIFO pipe drawing from one
                    # shared ~370GB/s pool, so per-queue order must match
                    # the matmul consumption schedule).
                    # First two fp8 pairs are per-pair DMAs so the very
                    # first matmuls start ~9us instead of waiting for a
                    # whole block (fine granularity costs ~1.4us of queue
                    # time per pair, so the rest ride batches).
                    third = NBF // 3
                    for j in range(2):
                        ksl = slice(2 * j, 2 * j + 2)
                        nc.scalar.dma_start(x8_res[:, ksl, :H], x8_ap[:, ksl, :H])
                        nc.sync.dma_start(w8_t[:, ksl, :H], w8_ap[:, ksl, :H])
                    ksl = slice(4, NFP8)
                    nc.scalar.dma_start(x8_res[:, ksl, :H], x8_ap[:, ksl, :H])
                    nc.sync.dma_start(w8_t[:, ksl, :H], w8_ap[:, ksl, :H])
                    for b in range(3):
                        ksl = slice(b * third, (b + 1) * third)
                        nc.scalar.dma_start(xb_res[:, ksl, :H], xb_ap[:, ksl, :H])
                        nc.sync.dma_start(wb_t[:, ksl, :H], wb_ap[:, ksl, :H])
                    nc.scalar.dma_start(x8_res[:, :, H:], x8_ap[:, :, H:])
                    nc.sync.dma_start(w8_t[:, :, H:], w8_ap[:, :, H : 2 * H])
                    for b in range(3):
                        ksl = slice(b * third, (b + 1) * third)
                        nc.scalar.dma_start(xb_res[:, ksl, H:], xb_ap[:, ksl, H:])
                        nc.sync.dma_start(wb_t[:, ksl, H:], wb_ap[:, ksl, H : 2 * H])

                    # nt0 (cols 0:512): k-outer over two mj-groups of 4
                    for grp in range(2):
                        pss = [
                            pspool.tile(
                                [P, N_TILE],
                                mybir.dt.float32,
                                name=f"ps0_{grp}_{i}",
                                tag="ps",
                            )
                            for i in range(4)
                        ]
                        mjs = [grp * 4 + i for i in range(4)]
                        mm_steps(
                            [[pss[i][:]] for i in range(4)],
                            w8_t,
                            wb_t,
                            mjs,
                            [slice(0, N_TILE)],
                        )
                        for i in range(4):
                            evict(pss[i][:], mjs[i], 0)

                    # nt1 (cols 512:1024): mj-outer, x resident
                    for mj in range(mj_cnt):
                        ps1 = pspool.tile(
                            [P, N_TILE], mybir.dt.float32, name=f"ps1_{mj}", tag="ps"
                        )
                        mm_steps(
                            [[ps1[:]]], w8_t, wb_t, [mj], [slice(N_TILE, 2 * N_TILE)]
                        )
                        evict(ps1[:], mj, N_TILE)
                    continue
                nc.sync.dma_start(w8_t[:, :, :], w8_ap[:, :, nsl])
                nc.sync.dma_start(wb_t[:, :, :], wb_ap[:, :, nsl])
                for mj in range(mj_cnt):
                    last = g == np_cnt - 1 and mj == mj_cnt - 1
                    ps_a = pspool.tile(
                        [P, N_TILE], mybir.dt.float32, name=f"psa_{g}_{mj}", tag="ps"
                    )
                    if not last:
                        ps_b = pspool.tile(
                            [P, N_TILE], mybir.dt.float32, name=f"psb_{g}_{mj}", tag="ps"
                        )
                        mm_steps(
                            [[ps_a[:], ps_b[:]]],
                            w8_t,
                            wb_t,
                            [mj],
                            [slice(0, N_TILE), slice(N_TILE, 2 * N_TILE)],
                        )
                        evict(ps_a[:], mj, n0)
                        evict(ps_b[:], mj, n0 + N_TILE)
                    else:
                        # Kernel-tail drain: nt0 normally, then nt1 as two
                        # sequential half-width groups so the first half's
                        # copy + store complete under the second half's
                        # matmuls.
                        mm_steps([[ps_a[:]]], w8_t, wb_t, [mj], [slice(0, N_TILE)])
                        evict(ps_a[:], mj, n0)
                        ps_b = pspool.tile(
                            [P, N_TILE], mybir.dt.float32, name=f"psbl_{g}_{mj}", tag="ps"
                        )
                        o_t = opool.tile([P, N_TILE], mybir.dt.float32)
                        h = N_TILE // 2
                        for half in range(2):
                            hsl = slice(N_TILE + half * h, N_TILE + (half + 1) * h)
                            psl = slice(half * h, (half + 1) * h)
                            mm_steps(
                                [[ps_b]], w8_t, wb_t, [mj], [hsl], psl=psl
                            )
                            nc.vector.tensor_copy(out=o_t[:, psl], in_=ps_b[:, psl])
                            nc.gpsimd.dma_start(
                                out_ap[
                                    mj, :, n0 + N_TILE + half * h : n0 + N_TILE + (half + 1) * h
                                ],
                                o_t[:, psl],
                            )

    return nc


_CACHE: dict = {}


def _get_finalized_nc():
    nc = _CACHE.get("nc")
    if nc is None:
        nc = build_nc()
        nc.finalize()
        _CACHE["nc"] = nc
    return nc


def _host_prep(x: np.ndarray, weight: np.ndarray):
    """Pre-sign weights, split-cast x, K-major transposes.

    Returns (x8_global [8*KF8, mc] fp8, xb_global [8*(K-KF8), mc] bf16,
             w8T [KF8, N] fp8, wbT [K-KF8, N] bf16)."""
    mc = M // N_CORES
    wb = np.sign(weight).astype(np.float32, copy=False)
    wT8 = np.ascontiguousarray(wb[:, :KF8].astype(FP8).view(np.uint8).T).view(FP8)
    wTb = (
        np.ascontiguousarray(wb[:, KF8:].astype(BF16).view(np.uint16).T).view(BF16)
    )
    # x: per-core K-major shards (transpose via integer views: ml_dtypes
    # object paths are slow for strided copies).
    x8 = np.ascontiguousarray(
        x[:, :KF8].astype(FP8).view(np.uint8).reshape(N_CORES, mc, KF8).transpose(0, 2, 1)
    )
    x8_global = x8.reshape(N_CORES * KF8, mc).view(FP8)
    xb = np.ascontiguousarray(
        x[:, KF8:].astype(BF16).view(np.uint16).reshape(N_CORES, mc, K - KF8).transpose(0, 2, 1)
    )
    xb_global = xb.reshape(N_CORES * (K - KF8), mc).view(BF16)
    return x8_global, xb_global, wT8, wTb


def make_in_maps(x: np.ndarray, weight: np.ndarray):
    x8_global, xb_global, wT8, wTb = _host_prep(x, weight)
    kb = K - KF8
    return [
        {
            "x8T": x8_global[c * KF8 : (c + 1) * KF8],
            "xbT": xb_global[c * kb : (c + 1) * kb],
            "w8T": wT8,
            "wbT": wTb,
        }
        for c in range(N_CORES)
    ]


def kernel(x: np.ndarray, weight: np.ndarray) -> np.ndarray:
    x = np.asarray(x)
    weight = np.asarray(weight)
    assert x.shape == (M, K) and weight.shape == (N, K)

    nc = _get_finalized_nc()
    from concourse.bass_utils import run_bass_kernel_spmd

    in_maps = make_in_maps(x, weight)
    try:
        res = run_bass_kernel_spmd(nc, in_maps, core_ids=list(range(N_CORES)))
    except Exception:
        # Transient device hiccups (e.g. NRT_EXEC_UNIT_UNRECOVERABLE) have
        # been observed once across many runs; one retry clears them.
        res = run_bass_kernel_spmd(nc, in_maps, core_ids=list(range(N_CORES)))
    out = np.concatenate([res.results[c]["out"] for c in range(N_CORES)], axis=0)
    return np.ascontiguousarray(out.astype(np.float32, copy=False))


# revision 14
# speedup vs baseline: 1.2117x; 1.1967x over previous
"""Binarized linear kernel for Trainium2 (8 NeuronCores, SPMD).

Computes out = x @ sign(weight).T with
  x:      [8192, 4096] f32
  weight: [4096, 4096] f32
  out:    [8192, 4096] f32

Strategy (data-parallel over M; mixed-precision contraction):
  - sign(weight) is exactly representable in fp8-e4m3 and bf16, so the
    weight binarization happens on the host for free and the device just
    streams pre-signed weights.
  - The PE's fp8 DoubleRow mode contracts 256 rows per 512-cycle matmul
    (2x the bf16 rate). Quantizing all of x to e4m3 costs 2.66e-2 rel
    error (over the 2e-2 budget), so only NFP8 of the 32 k-chunks of
    128 use the fp8 path (x in e4m3) and the rest stay bf16: total rel
    err ~= 2.66e-2 * sqrt(NFP8/32), while PE time per output tile drops
    from 32 to NFP8/2 (DoubleRow pairs) + (32-NFP8) matmuls.
  - Each core keeps its x shard resident in SBUF (fp8 + bf16 copies,
    K-major so the contraction dim is on partitions) and streams the
    shared pre-signed weights once.
  - DMA issue is the ramp bottleneck (each DMA instruction costs ~0.6us
    on its issuing engine), so loads are batched into few multi-chunk
    3D DMAs split across the two hardware-DGE queues (Sync carries w,
    Scalar carries x) while output stores ride Sync in steady state.
  - Ramp: 20 dummy warm-up matmuls un-throttle the HAM clock gate while
    the first DMAs land; the first n-tile runs k-outer over mj-groups
    of 4 (half-m x loads, half-width w loads) so matmuls chase the DMA
    arrival order; the very last psum bank is drained as two half-width
    accumulation groups to shrink the tail.
"""

import os
import sys

import numpy as np

# Toolchain locations (normally already on sys.path via PYTHONPATH; be
# robust when invoked from a fresh directory/environment).
for _p in (
    "/root/.axon_site",
    "/root/.axon_site/_ro/trn_rl_repo",
    "/root/.axon_site/_ro/pypackages",
    "/opt/trn_rl_repo",
):
    if os.path.isdir(_p) and _p not in sys.path:
        sys.path.append(_p)

import ml_dtypes  # noqa: E402

BF16 = ml_dtypes.bfloat16
FP8 = ml_dtypes.float8_e4m3  # IEEE-style e4m3: matches TRN FP8_EXP4 for |v|<=240

M, K, N = 8192, 4096, 4096
N_CORES = 8
P = 128
N_TILE = 512

NFP8 = 14  # k-chunks (of 128) contracted in fp8 DoubleRow; must be even
NBF = K // P - NFP8  # k-chunks contracted in bf16
KF8 = NFP8 * P  # leading k rows in fp8


def build_nc(mc: int = M // N_CORES, n: int = N):
    """Per-core Bass program (SPMD: same program, different data)."""
    from concourse import bacc, mybir, tile

    DR = mybir.MatmulPerfMode.DoubleRow
    npair = NFP8 // 2
    mj_cnt = mc // P
    np_cnt = n // (2 * N_TILE)  # n-tile PAIRS of 1024 columns

    nc = bacc.Bacc("TRN2", target_bir_lowering=False)

    x8T = nc.dram_tensor("x8T", [KF8, mc], mybir.dt.float8e4, kind="ExternalInput")
    xbT = nc.dram_tensor("xbT", [K - KF8, mc], mybir.dt.bfloat16, kind="ExternalInput")
    w8T = nc.dram_tensor("w8T", [KF8, n], mybir.dt.float8e4, kind="ExternalInput")
    wbT = nc.dram_tensor("wbT", [K - KF8, n], mybir.dt.bfloat16, kind="ExternalInput")
    out = nc.dram_tensor("out", [mc, n], mybir.dt.float32, kind="ExternalOutput")

    x8_ap = x8T[:].rearrange("(ko p) m -> p ko m", p=P)
    xb_ap = xbT[:].rearrange("(ko p) m -> p ko m", p=P)
    w8_ap = w8T[:].rearrange("(ko p) n -> p ko n", p=P)
    wb_ap = wbT[:].rearrange("(ko p) n -> p ko n", p=P)
    out_ap = out[:].rearrange("(t p) n -> t p n", p=P)

    H = N_TILE  # 512

    with tile.TileContext(nc) as tc:
        with (
            tc.tile_pool(name="xres", bufs=1) as xpool,
            tc.tile_pool(name="warmp", bufs=1) as warmpool,
            tc.tile_pool(name="w", bufs=2) as wpool,
            tc.tile_pool(name="o", bufs=6) as opool,
            tc.tile_pool(name="ps", bufs=8, space="PSUM") as pspool,
        ):
            # HAM warm-up: dummy matmuls on a zeroed tile fill the dead
            # PE time while the prologue + first DMAs run, so the real
            # matmul stream starts at 2.4 GHz instead of 1.2.
            warm = warmpool.tile([P, N_TILE], mybir.dt.bfloat16)
            nc.vector.memset(warm[:], 0)
            warm_ps = pspool.tile([P, N_TILE], mybir.dt.float32, tag="ps")
            for _ in range(4):
                nc.tensor.matmul(warm_ps[:], warm[:, :P], warm[:], start=True, stop=True)

            x8_res = xpool.tile([P, NFP8, mc], mybir.dt.float8e4)
            xb_res = xpool.tile([P, NBF, mc], mybir.dt.bfloat16)

            def mm_steps(ps_list, w8_t, wb_t, mjs, nsl_w_list, psl=None):
                """Accumulate 25 chunk-steps into the psum tiles: one MM
                per (mj, n-slice) per step, fp8 DoubleRow pairs first
                (matching the ramp's DMA arrival order)."""
                for j in range(npair):
                    for i, mj in enumerate(mjs):
                        msl = slice(mj * P, (mj + 1) * P)
                        for t, ps in enumerate(ps_list[i]):
                            nc.tensor.matmul(
                                ps if psl is None else ps[:, psl],
                                x8_res[:, 2 * j : 2 * j + 2, msl],
                                w8_t[:, 2 * j : 2 * j + 2, nsl_w_list[t]],
                                start=(j == 0),
                                stop=False,
                                perf_mode=DR,
                            )
                for ko in range(NBF):
                    for i, mj in enumerate(mjs):
                        msl = slice(mj * P, (mj + 1) * P)
                        for t, ps in enumerate(ps_list[i]):
                            nc.tensor.matmul(
                                ps if psl is None else ps[:, psl],
                                xb_res[:, ko, msl],
                                wb_t[:, ko, nsl_w_list[t]],
                                start=False,
                                stop=(ko == NBF - 1),
                            )

            def evict(ps_ap, mj, n0, width=N_TILE):
                # Stores ride the GpSimd SWDGE queue so they never
                # head-of-line-block the w prefetches on Sync.
                o_t = opool.tile([P, N_TILE], mybir.dt.float32, name="o_t")
                nc.vector.tensor_copy(out=o_t[:, :width], in_=ps_ap)
                nc.gpsimd.dma_start(out_ap[mj, :, n0 : n0 + width], o_t[:, :width])

            # ---- n-pairs 0..3. Pair 0 is the ramp: its loads are split
            # into halves across the two hardware-DGE queues (x on Scalar,
            # w on Sync) in consumption order, and its compute runs nt0
            # k-outer (two mj-groups of 4) then nt1 mj-outer, chasing the
            # DMA arrivals. Pairs 1..3 use batched w loads (one DMA per
            # dtype, prefetched a full pair ahead) and paired mj-outer.
            for g in range(np_cnt):
                w8_t = wpool.tile([P, NFP8, 2 * N_TILE], mybir.dt.float8e4, name="w8_t")
                wb_t = wpool.tile([P, NBF, 2 * N_TILE], mybir.dt.bfloat16, name="wb_t")
                n0 = g * 2 * N_TILE
                nsl = slice(n0, n0 + 2 * N_TILE)
                if g == 0:
                    # Ramp loads: sub-batched DMAs in consumption order,
                    # x on Scalar / w on Sync (the two hardware-DGE
                    # queues; each queue is a FIFO pipe drawing from one
                    # shared ~370GB/s pool, so per-queue order must match
                    # the matmul consumption schedule).
                    # First two fp8 pairs are per-pair DMAs so the very
                    # first matmuls start ~9us instead of waiting for a
                    # whole block (fine granularity costs ~1.4us of queue
                    # time per pair, so the rest ride batches).
                    third = NBF // 3
                    for j in range(3):
                        ksl = slice(2 * j, 2 * j + 2)
                        nc.scalar.dma_start(x8_res[:, ksl, :H], x8_ap[:, ksl, :H])
                        nc.sync.dma_start(w8_t[:, ksl, :H], w8_ap[:, ksl, :H])
                    ksl = slice(6, NFP8)
                    nc.scalar.dma_start(x8_res[:, ksl, :H], x8_ap[:, ksl, :H])
                    nc.sync.dma_start(w8_t[:, ksl, :H], w8_ap[:, ksl, :H])
                    for b in range(3):
                        ksl = slice(b * third, (b + 1) * third)
                        nc.scalar.dma_start(xb_res[:, ksl, :H], xb_ap[:, ksl, :H])
                        nc.sync.dma_start(wb_t[:, ksl, :H], wb_ap[:, ksl, :H])
                    nc.scalar.dma_start(x8_res[:, :, H:], x8_ap[:, :, H:])
                    nc.sync.dma_start(w8_t[:, :, H:], w8_ap[:, :, H : 2 * H])
                    for b in range(3):
                        ksl = slice(b * third, (b + 1) * third)
                        nc.scalar.dma_start(xb_res[:, ksl, H:], xb_ap[:, ksl, H:])
                        nc.sync.dma_start(wb_t[:, ksl, H:], wb_ap[:, ksl, H : 2 * H])

                    # nt0 (cols 0:512): k-outer over two mj-groups of 4
                    for grp in range(2):
                        pss = [
                            pspool.tile(
                                [P, N_TILE],
                                mybir.dt.float32,
                                name=f"ps0_{grp}_{i}",
                                tag="ps",
                            )
                            for i in range(4)
                        ]
                        mjs = [grp * 4 + i for i in range(4)]
                        mm_steps(
                            [[pss[i][:]] for i in range(4)],
                            w8_t,
                            wb_t,
                            mjs,
                            [slice(0, N_TILE)],
                        )
                        for i in range(4):
                            evict(pss[i][:], mjs[i], 0)

                    # nt1 (cols 512:1024): mj-outer, x resident
                    for mj in range(mj_cnt):
                        ps1 = pspool.tile(
                            [P, N_TILE], mybir.dt.float32, name=f"ps1_{mj}", tag="ps"
                        )
                        mm_steps(
                            [[ps1[:]]], w8_t, wb_t, [mj], [slice(N_TILE, 2 * N_TILE)]
                        )
                        evict(ps1[:], mj, N_TILE)
                    continue
                nc.sync.dma_start(w8_t[:, :, :], w8_ap[:, :, nsl])
                nc.sync.dma_start(wb_t[:, :, :], wb_ap[:, :, nsl])
                for mj in range(mj_cnt):
                    last = g == np_cnt - 1 and mj == mj_cnt - 1
                    ps_a = pspool.tile(
                        [P, N_TILE], mybir.dt.float32, name=f"psa_{g}_{mj}", tag="ps"
                    )
                    if not last:
                        ps_b = pspool.tile(
                            [P, N_TILE], mybir.dt.float32, name=f"psb_{g}_{mj}", tag="ps"
                        )
                        mm_steps(
                            [[ps_a[:], ps_b[:]]],
                            w8_t,
                            wb_t,
                            [mj],
                            [slice(0, N_TILE), slice(N_TILE, 2 * N_TILE)],
                        )
                        evict(ps_a[:], mj, n0)
                        evict(ps_b[:], mj, n0 + N_TILE)
                    else:
                        # Kernel-tail drain: nt0 normally, then nt1 as two
                        # sequential half-width groups so the first half's
                        # copy + store complete under the second half's
                        # matmuls.
                        mm_steps([[ps_a[:]]], w8_t, wb_t, [mj], [slice(0, N_TILE)])
                        evict(ps_a[:], mj, n0)
                        ps_b = pspool.tile(
                            [P, N_TILE], mybir.dt.float32, name=f"psbl_{g}_{mj}", tag="ps"
                        )
                        o_t = opool.tile([P, N_TILE], mybir.dt.float32)
                        h = N_TILE // 2
                        for half in range(2):
                            hsl = slice(N_TILE + half * h, N_TILE + (half + 1) * h)
                            psl = slice(half * h, (half + 1) * h)
                            mm_steps(
                                [[ps_b]], w8_t, wb_t, [mj], [hsl], psl=psl
                            )
                            nc.vector.tensor_copy(out=o_t[:, psl], in_=ps_b[:, psl])
                            nc.gpsimd.dma_start(
                                out_ap[
                                    mj, :, n0 + N_TILE + half * h : n0 + N_TILE + (half + 1) * h
                                ],
                                o_t[:, psl],
                            )

    return nc


_CACHE: dict = {}


def _get_finalized_nc():
    nc = _CACHE.get("nc")
    if nc is None:
        nc = build_nc()
        nc.finalize()
        _CACHE["nc"] = nc
    return nc


def _host_prep(x: np.ndarray, weight: np.ndarray):
    """Pre-sign weights, split-cast x, K-major transposes.

    Returns (x8_global [8*KF8, mc] fp8, xb_global [8*(K-KF8), mc] bf16,
             w8T [KF8, N] fp8, wbT [K-KF8, N] bf16)."""
    mc = M // N_CORES
    wb = np.sign(weight).astype(np.float32, copy=False)
    wT8 = np.ascontiguousarray(wb[:, :KF8].astype(FP8).view(np.uint8).T).view(FP8)
    wTb = (
        np.ascontiguousarray(wb[:, KF8:].astype(BF16).view(np.uint16).T).view(BF16)
    )
    # x: per-core K-major shards (transpose via integer views: ml_dtypes
    # object paths are slow for strided copies).
    x8 = np.ascontiguousarray(
        x[:, :KF8].astype(FP8).view(np.uint8).reshape(N_CORES, mc, KF8).transpose(0, 2, 1)
    )
    x8_global = x8.reshape(N_CORES * KF8, mc).view(FP8)
    xb = np.ascontiguousarray(
        x[:, KF8:].astype(BF16).view(np.uint16).reshape(N_CORES, mc, K - KF8).transpose(0, 2, 1)
    )
    xb_global = xb.reshape(N_CORES * (K - KF8), mc).view(BF16)
    return x8_global, xb_global, wT8, wTb


def make_in_maps(x: np.ndarray, weight: np.ndarray):
    x8_global, xb_global, wT8, wTb = _host_prep(x, weight)
    kb = K - KF8
    return [
        {
            "x8T": x8_global[c * KF8 : (c + 1) * KF8],
            "xbT": xb_global[c * kb : (c + 1) * kb],
            "w8T": wT8,
            "wbT": wTb,
        }
        for c in range(N_CORES)
    ]


def kernel(x: np.ndarray, weight: np.ndarray) -> np.ndarray:
    x = np.asarray(x)
    weight = np.asarray(weight)
    assert x.shape == (M, K) and weight.shape == (N, K)

    nc = _get_finalized_nc()
    from concourse.bass_utils import run_bass_kernel_spmd

    in_maps = make_in_maps(x, weight)
    try:
        res = run_bass_kernel_spmd(nc, in_maps, core_ids=list(range(N_CORES)))
    except Exception:
        # Transient device hiccups (e.g. NRT_EXEC_UNIT_UNRECOVERABLE) have
        # been observed once across many runs; one retry clears them.
        res = run_bass_kernel_spmd(nc, in_maps, core_ids=list(range(N_CORES)))
    out = np.concatenate([res.results[c]["out"] for c in range(N_CORES)], axis=0)
    return np.ascontiguousarray(out.astype(np.float32, copy=False))


# revision 16
# speedup vs baseline: 1.2649x; 1.0439x over previous
"""Binarized linear kernel for Trainium2 (8 NeuronCores, SPMD).

Computes out = x @ sign(weight).T with
  x:      [8192, 4096] f32
  weight: [4096, 4096] f32
  out:    [8192, 4096] f32

Strategy (data-parallel over M; mixed-precision contraction):
  - sign(weight) is exactly representable in fp8-e4m3 and bf16, so the
    weight binarization happens on the host for free and the device just
    streams pre-signed weights.
  - The PE's fp8 DoubleRow mode contracts 256 rows per 512-cycle matmul
    (2x the bf16 rate). Quantizing all of x to e4m3 costs 2.66e-2 rel
    error (over the 2e-2 budget), so only NFP8 of the 32 k-chunks of
    128 use the fp8 path (x in e4m3) and the rest stay bf16: total rel
    err ~= 2.66e-2 * sqrt(NFP8/32), while PE time per output tile drops
    from 32 to NFP8/2 (DoubleRow pairs) + (32-NFP8) matmuls.
  - Each core keeps its x shard resident in SBUF (fp8 + bf16 copies,
    K-major so the contraction dim is on partitions) and streams the
    shared pre-signed weights once.
  - DMA issue is the ramp bottleneck (each DMA instruction costs ~0.6us
    on its issuing engine), so loads are batched into few multi-chunk
    3D DMAs split across the two hardware-DGE queues (Sync carries w,
    Scalar carries x) while output stores ride Sync in steady state.
  - Ramp: 20 dummy warm-up matmuls un-throttle the HAM clock gate while
    the first DMAs land; the first n-tile runs k-outer over mj-groups
    of 4 (half-m x loads, half-width w loads) so matmuls chase the DMA
    arrival order; the very last psum bank is drained as two half-width
    accumulation groups to shrink the tail.
"""

import os
import sys

import numpy as np

# Toolchain locations (normally already on sys.path via PYTHONPATH; be
# robust when invoked from a fresh directory/environment).
for _p in (
    "/root/.axon_site",
    "/root/.axon_site/_ro/trn_rl_repo",
    "/root/.axon_site/_ro/pypackages",
    "/opt/trn_rl_repo",
):
    if os.path.isdir(_p) and _p not in sys.path:
        sys.path.append(_p)

import ml_dtypes  # noqa: E402

BF16 = ml_dtypes.bfloat16
FP8 = ml_dtypes.float8_e4m3  # IEEE-style e4m3: matches TRN FP8_EXP4 for |v|<=240

M, K, N = 8192, 4096, 4096
N_CORES = 8
P = 128
N_TILE = 512

NFP8 = 16  # k-chunks (of 128) contracted in fp8 DoubleRow; must be even
NBF = K // P - NFP8  # k-chunks contracted in bf16
KF8 = NFP8 * P  # leading k rows in fp8


def build_nc(mc: int = M // N_CORES, n: int = N):
    """Per-core Bass program (SPMD: same program, different data)."""
    from concourse import bacc, mybir, tile

    DR = mybir.MatmulPerfMode.DoubleRow
    npair = NFP8 // 2
    mj_cnt = mc // P
    np_cnt = n // (2 * N_TILE)  # n-tile PAIRS of 1024 columns

    nc = bacc.Bacc("TRN2", target_bir_lowering=False)

    x8T = nc.dram_tensor("x8T", [KF8, mc], mybir.dt.float8e4, kind="ExternalInput")
    xbT = nc.dram_tensor("xbT", [K - KF8, mc], mybir.dt.bfloat16, kind="ExternalInput")
    w8T = nc.dram_tensor("w8T", [KF8, n], mybir.dt.float8e4, kind="ExternalInput")
    wbT = nc.dram_tensor("wbT", [K - KF8, n], mybir.dt.bfloat16, kind="ExternalInput")
    out = nc.dram_tensor("out", [mc, n], mybir.dt.float32, kind="ExternalOutput")

    x8_ap = x8T[:].rearrange("(ko p) m -> p ko m", p=P)
    xb_ap = xbT[:].rearrange("(ko p) m -> p ko m", p=P)
    w8_ap = w8T[:].rearrange("(ko p) n -> p ko n", p=P)
    wb_ap = wbT[:].rearrange("(ko p) n -> p ko n", p=P)
    out_ap = out[:].rearrange("(t p) n -> t p n", p=P)

    H = N_TILE  # 512

    with tile.TileContext(nc) as tc:
        with (
            tc.tile_pool(name="xres", bufs=1) as xpool,
            tc.tile_pool(name="warmp", bufs=1) as warmpool,
            tc.tile_pool(name="w", bufs=2) as wpool,
            tc.tile_pool(name="o", bufs=6) as opool,
            tc.tile_pool(name="ps", bufs=8, space="PSUM") as pspool,
        ):
            # HAM warm-up: dummy matmuls on a zeroed tile fill the dead
            # PE time while the prologue + first DMAs run, so the real
            # matmul stream starts at 2.4 GHz instead of 1.2.
            warm = warmpool.tile([P, N_TILE], mybir.dt.bfloat16)
            nc.vector.memset(warm[:], 0)
            warm_ps = pspool.tile([P, N_TILE], mybir.dt.float32, tag="ps")
            for _ in range(4):
                nc.tensor.matmul(warm_ps[:], warm[:, :P], warm[:], start=True, stop=True)

            x8_res = xpool.tile([P, NFP8, mc], mybir.dt.float8e4)
            xb_res = xpool.tile([P, NBF, mc], mybir.dt.bfloat16)

            def mm_steps(ps_list, w8_t, wb_t, mjs, nsl_w_list, psl=None):
                """Accumulate 25 chunk-steps into the psum tiles: one MM
                per (mj, n-slice) per step, fp8 DoubleRow pairs first
                (matching the ramp's DMA arrival order)."""
                for j in range(npair):
                    for i, mj in enumerate(mjs):
                        msl = slice(mj * P, (mj + 1) * P)
                        for t, ps in enumerate(ps_list[i]):
                            nc.tensor.matmul(
                                ps if psl is None else ps[:, psl],
                                x8_res[:, 2 * j : 2 * j + 2, msl],
                                w8_t[:, 2 * j : 2 * j + 2, nsl_w_list[t]],
                                start=(j == 0),
                                stop=False,
                                perf_mode=DR,
                            )
                for ko in range(NBF):
                    for i, mj in enumerate(mjs):
                        msl = slice(mj * P, (mj + 1) * P)
                        for t, ps in enumerate(ps_list[i]):
                            nc.tensor.matmul(
                                ps if psl is None else ps[:, psl],
                                xb_res[:, ko, msl],
                                wb_t[:, ko, nsl_w_list[t]],
                                start=False,
                                stop=(ko == NBF - 1),
                            )

            def evict(ps_ap, mj, n0, width=N_TILE):
                o_t = opool.tile([P, N_TILE], mybir.dt.float32, name="o_t")
                nc.vector.tensor_copy(out=o_t[:, :width], in_=ps_ap)
                nc.sync.dma_start(out_ap[mj, :, n0 : n0 + width], o_t[:, :width])

            # ---- n-pairs 0..3. Pair 0 is the ramp: its loads are split
            # into halves across the two hardware-DGE queues (x on Scalar,
            # w on Sync) in consumption order, and its compute runs nt0
            # k-outer (two mj-groups of 4) then nt1 mj-outer, chasing the
            # DMA arrivals. Pairs 1..3 use batched w loads (one DMA per
            # dtype, prefetched a full pair ahead) and paired mj-outer.
            for g in range(np_cnt):
                w8_t = wpool.tile([P, NFP8, 2 * N_TILE], mybir.dt.float8e4, name="w8_t")
                wb_t = wpool.tile([P, NBF, 2 * N_TILE], mybir.dt.bfloat16, name="wb_t")
                n0 = g * 2 * N_TILE
                nsl = slice(n0, n0 + 2 * N_TILE)
                if g == 0:
                    # Ramp loads: sub-batched DMAs in consumption order,
                    # x on Scalar / w on Sync (the two hardware-DGE
                    # queues; each queue is a FIFO pipe drawing from one
                    # shared ~370GB/s pool, so per-queue order must match
                    # the matmul consumption schedule).
                    # First two fp8 pairs are per-pair DMAs so the very
                    # first matmuls start ~9us instead of waiting for a
                    # whole block (fine granularity costs ~1.4us of queue
                    # time per pair, so the rest ride batches).
                    tb = [0, NBF // 3, (2 * NBF) // 3, NBF]
                    for j in range(3):
                        ksl = slice(2 * j, 2 * j + 2)
                        nc.scalar.dma_start(x8_res[:, ksl, :H], x8_ap[:, ksl, :H])
                        nc.sync.dma_start(w8_t[:, ksl, :H], w8_ap[:, ksl, :H])
                    ksl = slice(6, NFP8)
                    nc.scalar.dma_start(x8_res[:, ksl, :H], x8_ap[:, ksl, :H])
                    nc.sync.dma_start(w8_t[:, ksl, :H], w8_ap[:, ksl, :H])
                    for b in range(3):
                        ksl = slice(tb[b], tb[b + 1])
                        nc.scalar.dma_start(xb_res[:, ksl, :H], xb_ap[:, ksl, :H])
                        nc.sync.dma_start(wb_t[:, ksl, :H], wb_ap[:, ksl, :H])
                    nc.scalar.dma_start(x8_res[:, :, H:], x8_ap[:, :, H:])
                    nc.sync.dma_start(w8_t[:, :, H:], w8_ap[:, :, H : 2 * H])
                    for b in range(3):
                        ksl = slice(tb[b], tb[b + 1])
                        nc.scalar.dma_start(xb_res[:, ksl, H:], xb_ap[:, ksl, H:])
                        nc.sync.dma_start(wb_t[:, ksl, H:], wb_ap[:, ksl, H : 2 * H])

                    # nt0 (cols 0:512): k-outer over two mj-groups of 4
                    for grp in range(2):
                        pss = [
                            pspool.tile(
                                [P, N_TILE],
                                mybir.dt.float32,
                                name=f"ps0_{grp}_{i}",
                                tag="ps",
                            )
                            for i in range(4)
                        ]
                        mjs = [grp * 4 + i for i in range(4)]
                        mm_steps(
                            [[pss[i][:]] for i in range(4)],
                            w8_t,
                            wb_t,
                            mjs,
                            [slice(0, N_TILE)],
                        )
                        for i in range(4):
                            evict(pss[i][:], mjs[i], 0)

                    # nt1 (cols 512:1024): mj-outer, x resident
                    for mj in range(mj_cnt):
                        ps1 = pspool.tile(
                            [P, N_TILE], mybir.dt.float32, name=f"ps1_{mj}", tag="ps"
                        )
                        mm_steps(
                            [[ps1[:]]], w8_t, wb_t, [mj], [slice(N_TILE, 2 * N_TILE)]
                        )
                        evict(ps1[:], mj, N_TILE)
                    continue
                nc.sync.dma_start(w8_t[:, :, :], w8_ap[:, :, nsl])
                nc.sync.dma_start(wb_t[:, :, :], wb_ap[:, :, nsl])
                for mj in range(mj_cnt):
                    last = g == np_cnt - 1 and mj == mj_cnt - 1
                    ps_a = pspool.tile(
                        [P, N_TILE], mybir.dt.float32, name=f"psa_{g}_{mj}", tag="ps"
                    )
                    if not last:
                        ps_b = pspool.tile(
                            [P, N_TILE], mybir.dt.float32, name=f"psb_{g}_{mj}", tag="ps"
                        )
                        mm_steps(
                            [[ps_a[:], ps_b[:]]],
                            w8_t,
                            wb_t,
                            [mj],
                            [slice(0, N_TILE), slice(N_TILE, 2 * N_TILE)],
                        )
                        evict(ps_a[:], mj, n0)
                        evict(ps_b[:], mj, n0 + N_TILE)
                    else:
                        # Kernel-tail drain: nt0 normally, then nt1 as two
                        # sequential half-width groups so the first half's
                        # copy + store complete under the second half's
                        # matmuls.
                        mm_steps([[ps_a[:]]], w8_t, wb_t, [mj], [slice(0, N_TILE)])
                        evict(ps_a[:], mj, n0)
                        ps_b = pspool.tile(
                            [P, N_TILE], mybir.dt.float32, name=f"psbl_{g}_{mj}", tag="ps"
                        )
                        o_t = opool.tile([P, N_TILE], mybir.dt.float32)
                        h = N_TILE // 2
                        for half in range(2):
                            hsl = slice(N_TILE + half * h, N_TILE + (half + 1) * h)
                            psl = slice(half * h, (half + 1) * h)
                            mm_steps(
                                [[ps_b]], w8_t, wb_t, [mj], [hsl], psl=psl
                            )
                            nc.vector.tensor_copy(out=o_t[:, psl], in_=ps_b[:, psl])
                            nc.sync.dma_start(
                                out_ap[
                                    mj, :, n0 + N_TILE + half * h : n0 + N_TILE + (half + 1) * h
                                ],
                                o_t[:, psl],
                            )

    return nc


_CACHE: dict = {}


def _get_finalized_nc():
    nc = _CACHE.get("nc")
    if nc is None:
        nc = build_nc()
        nc.finalize()
        _CACHE["nc"] = nc
    return nc


def _host_prep(x: np.ndarray, weight: np.ndarray):
    """Pre-sign weights, split-cast x, K-major transposes.

    Returns (x8_global [8*KF8, mc] fp8, xb_global [8*(K-KF8), mc] bf16,
             w8T [KF8, N] fp8, wbT [K-KF8, N] bf16)."""
    mc = M // N_CORES
    wb = np.sign(weight).astype(np.float32, copy=False)
    wT8 = np.ascontiguousarray(wb[:, :KF8].astype(FP8).view(np.uint8).T).view(FP8)
    wTb = (
        np.ascontiguousarray(wb[:, KF8:].astype(BF16).view(np.uint16).T).view(BF16)
    )
    # x: per-core K-major shards (transpose via integer views: ml_dtypes
    # object paths are slow for strided copies).
    x8 = np.ascontiguousarray(
        x[:, :KF8].astype(FP8).view(np.uint8).reshape(N_CORES, mc, KF8).transpose(0, 2, 1)
    )
    x8_global = x8.reshape(N_CORES * KF8, mc).view(FP8)
    xb = np.ascontiguousarray(
        x[:, KF8:].astype(BF16).view(np.uint16).reshape(N_CORES, mc, K - KF8).transpose(0, 2, 1)
    )
    xb_global = xb.reshape(N_CORES * (K - KF8), mc).view(BF16)
    return x8_global, xb_global, wT8, wTb


def make_in_maps(x: np.ndarray, weight: np.ndarray):
    x8_global, xb_global, wT8, wTb = _host_prep(x, weight)
    kb = K - KF8
    return [
        {
            "x8T": x8_global[c * KF8 : (c + 1) * KF8],
            "xbT": xb_global[c * kb : (c + 1) * kb],
            "w8T": wT8,
            "wbT": wTb,
        }
        for c in range(N_CORES)
    ]


def kernel(x: np.ndarray, weight: np.ndarray) -> np.ndarray:
    x = np.asarray(x)
    weight = np.asarray(weight)
    assert x.shape == (M, K) and weight.shape == (N, K)

    nc = _get_finalized_nc()
    from concourse.bass_utils import run_bass_kernel_spmd

    in_maps = make_in_maps(x, weight)
    try:
        res = run_bass_kernel_spmd(nc, in_maps, core_ids=list(range(N_CORES)))
    except Exception:
        # Transient device hiccups (e.g. NRT_EXEC_UNIT_UNRECOVERABLE) have
        # been observed once across many runs; one retry clears them.
        res = run_bass_kernel_spmd(nc, in_maps, core_ids=list(range(N_CORES)))
    out = np.concatenate([res.results[c]["out"] for c in range(N_CORES)], axis=0)
    return np.ascontiguousarray(out.astype(np.float32, copy=False))


# revision 17
# speedup vs baseline: 1.2768x; 1.0094x over previous
"""Binarized linear kernel for Trainium2 (8 NeuronCores, SPMD).

Computes out = x @ sign(weight).T with
  x:      [8192, 4096] f32
  weight: [4096, 4096] f32
  out:    [8192, 4096] f32

Strategy (data-parallel over M; mixed-precision contraction):
  - sign(weight) is exactly representable in fp8-e4m3 and bf16, so the
    weight binarization happens on the host for free and the device just
    streams pre-signed weights.
  - The PE's fp8 DoubleRow mode contracts 256 rows per 512-cycle matmul
    (2x the bf16 rate). Quantizing all of x to e4m3 costs 2.66e-2 rel
    error (over the 2e-2 budget), so only NFP8 of the 32 k-chunks of
    128 use the fp8 path (x in e4m3) and the rest stay bf16: total rel
    err ~= 2.66e-2 * sqrt(NFP8/32), while PE time per output tile drops
    from 32 to NFP8/2 (DoubleRow pairs) + (32-NFP8) matmuls.
  - Each core keeps its x shard resident in SBUF (fp8 + bf16 copies,
    K-major so the contraction dim is on partitions) and streams the
    shared pre-signed weights once.
  - DMA issue is the ramp bottleneck (each DMA instruction costs ~0.6us
    on its issuing engine), so loads are batched into few multi-chunk
    3D DMAs split across the two hardware-DGE queues (Sync carries w,
    Scalar carries x) while output stores ride Sync in steady state.
  - Ramp: 20 dummy warm-up matmuls un-throttle the HAM clock gate while
    the first DMAs land; the first n-tile runs k-outer over mj-groups
    of 4 (half-m x loads, half-width w loads) so matmuls chase the DMA
    arrival order; the very last psum bank is drained as two half-width
    accumulation groups to shrink the tail.
"""

import os
import sys

import numpy as np

# Toolchain locations (normally already on sys.path via PYTHONPATH; be
# robust when invoked from a fresh directory/environment).
for _p in (
    "/root/.axon_site",
    "/root/.axon_site/_ro/trn_rl_repo",
    "/root/.axon_site/_ro/pypackages",
    "/opt/trn_rl_repo",
):
    if os.path.isdir(_p) and _p not in sys.path:
        sys.path.append(_p)

import ml_dtypes  # noqa: E402

BF16 = ml_dtypes.bfloat16
FP8 = ml_dtypes.float8_e4m3  # IEEE-style e4m3: matches TRN FP8_EXP4 for |v|<=240

M, K, N = 8192, 4096, 4096
N_CORES = 8
P = 128
N_TILE = 512

NFP8 = 16  # k-chunks (of 128) contracted in fp8 DoubleRow; must be even
NBF = K // P - NFP8  # k-chunks contracted in bf16
KF8 = NFP8 * P  # leading k rows in fp8


def build_nc(mc: int = M // N_CORES, n: int = N):
    """Per-core Bass program (SPMD: same program, different data)."""
    from concourse import bacc, mybir, tile

    DR = mybir.MatmulPerfMode.DoubleRow
    npair = NFP8 // 2
    mj_cnt = mc // P
    np_cnt = n // (2 * N_TILE)  # n-tile PAIRS of 1024 columns

    nc = bacc.Bacc("TRN2", target_bir_lowering=False)

    x8T = nc.dram_tensor("x8T", [KF8, mc], mybir.dt.float8e4, kind="ExternalInput")
    xbT = nc.dram_tensor("xbT", [K - KF8, mc], mybir.dt.bfloat16, kind="ExternalInput")
    w8T = nc.dram_tensor("w8T", [KF8, n], mybir.dt.float8e4, kind="ExternalInput")
    wbT = nc.dram_tensor("wbT", [K - KF8, n], mybir.dt.bfloat16, kind="ExternalInput")
    out = nc.dram_tensor("out", [mc, n], mybir.dt.float32, kind="ExternalOutput")

    x8_ap = x8T[:].rearrange("(ko p) m -> p ko m", p=P)
    xb_ap = xbT[:].rearrange("(ko p) m -> p ko m", p=P)
    w8_ap = w8T[:].rearrange("(ko p) n -> p ko n", p=P)
    wb_ap = wbT[:].rearrange("(ko p) n -> p ko n", p=P)
    out_ap = out[:].rearrange("(t p) n -> t p n", p=P)

    H = N_TILE  # 512

    with tile.TileContext(nc) as tc:
        with (
            tc.tile_pool(name="xres", bufs=1) as xpool,
            tc.tile_pool(name="warmp", bufs=1) as warmpool,
            tc.tile_pool(name="w", bufs=2) as wpool,
            tc.tile_pool(name="o", bufs=6) as opool,
            tc.tile_pool(name="ps", bufs=8, space="PSUM") as pspool,
        ):
            # HAM warm-up: dummy matmuls on a zeroed tile fill the dead
            # PE time while the prologue + first DMAs run, so the real
            # matmul stream starts at 2.4 GHz instead of 1.2.
            warm = warmpool.tile([P, N_TILE], mybir.dt.bfloat16)
            nc.vector.memset(warm[:], 0)
            warm_ps = pspool.tile([P, N_TILE], mybir.dt.float32, tag="ps")
            for _ in range(4):
                nc.tensor.matmul(warm_ps[:], warm[:, :P], warm[:], start=True, stop=True)

            x8_res = xpool.tile([P, NFP8, mc], mybir.dt.float8e4)
            xb_res = xpool.tile([P, NBF, mc], mybir.dt.bfloat16)

            def mm_steps(ps_list, w8_t, wb_t, mjs, nsl_w_list, psl=None):
                """Accumulate 25 chunk-steps into the psum tiles: one MM
                per (mj, n-slice) per step, fp8 DoubleRow pairs first
                (matching the ramp's DMA arrival order)."""
                for j in range(npair):
                    for i, mj in enumerate(mjs):
                        msl = slice(mj * P, (mj + 1) * P)
                        for t, ps in enumerate(ps_list[i]):
                            nc.tensor.matmul(
                                ps if psl is None else ps[:, psl],
                                x8_res[:, 2 * j : 2 * j + 2, msl],
                                w8_t[:, 2 * j : 2 * j + 2, nsl_w_list[t]],
                                start=(j == 0),
                                stop=False,
                                perf_mode=DR,
                            )
                for ko in range(NBF):
                    for i, mj in enumerate(mjs):
                        msl = slice(mj * P, (mj + 1) * P)
                        for t, ps in enumerate(ps_list[i]):
                            nc.tensor.matmul(
                                ps if psl is None else ps[:, psl],
                                xb_res[:, ko, msl],
                                wb_t[:, ko, nsl_w_list[t]],
                                start=False,
                                stop=(ko == NBF - 1),
                            )

            def evict(ps_ap, mj, n0, width=N_TILE):
                o_t = opool.tile([P, N_TILE], mybir.dt.float32, name="o_t")
                nc.vector.tensor_copy(out=o_t[:, :width], in_=ps_ap)
                nc.sync.dma_start(out_ap[mj, :, n0 : n0 + width], o_t[:, :width])

            # ---- n-pairs 0..3. Pair 0 is the ramp: its loads are split
            # into halves across the two hardware-DGE queues (x on Scalar,
            # w on Sync) in consumption order, and its compute runs nt0
            # k-outer (two mj-groups of 4) then nt1 mj-outer, chasing the
            # DMA arrivals. Pairs 1..3 use batched w loads (one DMA per
            # dtype, prefetched a full pair ahead) and paired mj-outer.
            for g in range(np_cnt):
                w8_t = wpool.tile([P, NFP8, 2 * N_TILE], mybir.dt.float8e4, name="w8_t")
                wb_t = wpool.tile([P, NBF, 2 * N_TILE], mybir.dt.bfloat16, name="wb_t")
                n0 = g * 2 * N_TILE
                nsl = slice(n0, n0 + 2 * N_TILE)
                if g == 0:
                    # Ramp loads: sub-batched DMAs in consumption order,
                    # x on Scalar / w on Sync (the two hardware-DGE
                    # queues; each queue is a FIFO pipe drawing from one
                    # shared ~370GB/s pool, so per-queue order must match
                    # the matmul consumption schedule).
                    # First two fp8 pairs are per-pair DMAs so the very
                    # first matmuls start ~9us instead of waiting for a
                    # whole block (fine granularity costs ~1.4us of queue
                    # time per pair, so the rest ride batches).
                    tb = [0, NBF // 3, (2 * NBF) // 3, NBF]
                    for j in range(2):
                        ksl = slice(2 * j, 2 * j + 2)
                        nc.scalar.dma_start(x8_res[:, ksl, :H], x8_ap[:, ksl, :H])
                        nc.sync.dma_start(w8_t[:, ksl, :H], w8_ap[:, ksl, :H])
                    for lo in range(4, NFP8, 6):
                        ksl = slice(lo, min(lo + 6, NFP8))
                        nc.scalar.dma_start(x8_res[:, ksl, :H], x8_ap[:, ksl, :H])
                        nc.sync.dma_start(w8_t[:, ksl, :H], w8_ap[:, ksl, :H])
                    for b in range(3):
                        ksl = slice(tb[b], tb[b + 1])
                        nc.scalar.dma_start(xb_res[:, ksl, :H], xb_ap[:, ksl, :H])
                        nc.sync.dma_start(wb_t[:, ksl, :H], wb_ap[:, ksl, :H])
                    nc.scalar.dma_start(x8_res[:, :, H:], x8_ap[:, :, H:])
                    nc.sync.dma_start(w8_t[:, :, H:], w8_ap[:, :, H : 2 * H])
                    for b in range(3):
                        ksl = slice(tb[b], tb[b + 1])
                        nc.scalar.dma_start(xb_res[:, ksl, H:], xb_ap[:, ksl, H:])
                        nc.sync.dma_start(wb_t[:, ksl, H:], wb_ap[:, ksl, H : 2 * H])

                    # nt0 (cols 0:512): k-outer over two mj-groups of 4
                    for grp in range(2):
                        pss = [
                            pspool.tile(
                                [P, N_TILE],
                                mybir.dt.float32,
                                name=f"ps0_{grp}_{i}",
                                tag="ps",
                            )
                            for i in range(4)
                        ]
                        mjs = [grp * 4 + i for i in range(4)]
                        mm_steps(
                            [[pss[i][:]] for i in range(4)],
                            w8_t,
                            wb_t,
                            mjs,
                            [slice(0, N_TILE)],
                        )
                        for i in range(4):
                            evict(pss[i][:], mjs[i], 0)

                    # nt1 (cols 512:1024): mj-outer, x resident
                    for mj in range(mj_cnt):
                        ps1 = pspool.tile(
                            [P, N_TILE], mybir.dt.float32, name=f"ps1_{mj}", tag="ps"
                        )
                        mm_steps(
                            [[ps1[:]]], w8_t, wb_t, [mj], [slice(N_TILE, 2 * N_TILE)]
                        )
                        evict(ps1[:], mj, N_TILE)
                    continue
                nc.sync.dma_start(w8_t[:, :, :], w8_ap[:, :, nsl])
                nc.sync.dma_start(wb_t[:, :, :], wb_ap[:, :, nsl])
                for mj in range(mj_cnt):
                    last = g == np_cnt - 1 and mj == mj_cnt - 1
                    ps_a = pspool.tile(
                        [P, N_TILE], mybir.dt.float32, name=f"psa_{g}_{mj}", tag="ps"
                    )
                    if not last:
                        ps_b = pspool.tile(
                            [P, N_TILE], mybir.dt.float32, name=f"psb_{g}_{mj}", tag="ps"
                        )
                        mm_steps(
                            [[ps_a[:], ps_b[:]]],
                            w8_t,
                            wb_t,
                            [mj],
                            [slice(0, N_TILE), slice(N_TILE, 2 * N_TILE)],
                        )
                        evict(ps_a[:], mj, n0)
                        evict(ps_b[:], mj, n0 + N_TILE)
                    else:
                        # Kernel-tail drain: nt0 normally, then nt1 as two
                        # sequential half-width groups so the first half's
                        # copy + store complete under the second half's
                        # matmuls.
                        mm_steps([[ps_a[:]]], w8_t, wb_t, [mj], [slice(0, N_TILE)])
                        evict(ps_a[:], mj, n0)
                        o_t = opool.tile([P, N_TILE], mybir.dt.float32)
                        h = N_TILE // 2
                        for half in range(2):
                            ps_b = pspool.tile(
                                [P, N_TILE],
                                mybir.dt.float32,
                                name=f"psbl_{g}_{mj}_{half}",
                                tag="ps",
                            )
                            hsl = slice(N_TILE + half * h, N_TILE + (half + 1) * h)
                            psl = slice(half * h, (half + 1) * h)
                            mm_steps(
                                [[ps_b]], w8_t, wb_t, [mj], [hsl], psl=psl
                            )
                            nc.vector.tensor_copy(out=o_t[:, psl], in_=ps_b[:, psl])
                            nc.sync.dma_start(
                                out_ap[
                                    mj, :, n0 + N_TILE + half * h : n0 + N_TILE + (half + 1) * h
                                ],
                                o_t[:, psl],
                            )

    return nc


_CACHE: dict = {}


def _get_finalized_nc():
    nc = _CACHE.get("nc")
    if nc is None:
        nc = build_nc()
        nc.finalize()
        _CACHE["nc"] = nc
    return nc


def _host_prep(x: np.ndarray, weight: np.ndarray):
    """Pre-sign weights, split-cast x, K-major transposes.

    Returns (x8_global [8*KF8, mc] fp8, xb_global [8*(K-KF8), mc] bf16,
             w8T [KF8, N] fp8, wbT [K-KF8, N] bf16)."""
    mc = M // N_CORES
    wb = np.sign(weight).astype(np.float32, copy=False)
    wT8 = np.ascontiguousarray(wb[:, :KF8].astype(FP8).view(np.uint8).T).view(FP8)
    wTb = (
        np.ascontiguousarray(wb[:, KF8:].astype(BF16).view(np.uint16).T).view(BF16)
    )
    # x: per-core K-major shards (transpose via integer views: ml_dtypes
    # object paths are slow for strided copies).
    x8 = np.ascontiguousarray(
        x[:, :KF8].astype(FP8).view(np.uint8).reshape(N_CORES, mc, KF8).transpose(0, 2, 1)
    )
    x8_global = x8.reshape(N_CORES * KF8, mc).view(FP8)
    xb = np.ascontiguousarray(
        x[:, KF8:].astype(BF16).view(np.uint16).reshape(N_CORES, mc, K - KF8).transpose(0, 2, 1)
    )
    xb_global = xb.reshape(N_CORES * (K - KF8), mc).view(BF16)
    return x8_global, xb_global, wT8, wTb


def make_in_maps(x: np.ndarray, weight: np.ndarray):
    x8_global, xb_global, wT8, wTb = _host_prep(x, weight)
    kb = K - KF8
    return [
        {
            "x8T": x8_global[c * KF8 : (c + 1) * KF8],
            "xbT": xb_global[c * kb : (c + 1) * kb],
            "w8T": wT8,
            "wbT": wTb,
        }
        for c in range(N_CORES)
    ]


def kernel(x: np.ndarray, weight: np.ndarray) -> np.ndarray:
    x = np.asarray(x)
    weight = np.asarray(weight)
    assert x.shape == (M, K) and weight.shape == (N, K)

    nc = _get_finalized_nc()
    from concourse.bass_utils import run_bass_kernel_spmd

    in_maps = make_in_maps(x, weight)
    try:
        res = run_bass_kernel_spmd(nc, in_maps, core_ids=list(range(N_CORES)))
    except Exception:
        # Transient device hiccups (e.g. NRT_EXEC_UNIT_UNRECOVERABLE) have
        # been observed once across many runs; one retry clears them.
        res = run_bass_kernel_spmd(nc, in_maps, core_ids=list(range(N_CORES)))
    out = np.concatenate([res.results[c]["out"] for c in range(N_CORES)], axis=0)
    return np.ascontiguousarray(out.astype(np.float32, copy=False))


# revision 18
# speedup vs baseline: 1.2815x; 1.0037x over previous
"""Binarized linear kernel for Trainium2 (8 NeuronCores, SPMD).

Computes out = x @ sign(weight).T with
  x:      [8192, 4096] f32
  weight: [4096, 4096] f32
  out:    [8192, 4096] f32

Strategy (data-parallel over M; mixed-precision contraction):
  - sign(weight) is exactly representable in fp8-e4m3 and bf16, so the
    weight binarization happens on the host for free and the device just
    streams pre-signed weights.
  - The PE's fp8 DoubleRow mode contracts 256 rows per 512-cycle matmul
    (2x the bf16 rate). Quantizing all of x to e4m3 costs 2.66e-2 rel
    error (over the 2e-2 budget), so only NFP8 of the 32 k-chunks of
    128 use the fp8 path (x in e4m3) and the rest stay bf16: total rel
    err ~= 2.66e-2 * sqrt(NFP8/32), while PE time per output tile drops
    from 32 to NFP8/2 (DoubleRow pairs) + (32-NFP8) matmuls.
  - Each core keeps its x shard resident in SBUF (fp8 + bf16 copies,
    K-major so the contraction dim is on partitions) and streams the
    shared pre-signed weights once.
  - DMA issue is the ramp bottleneck (each DMA instruction costs ~0.6us
    on its issuing engine), so loads are batched into few multi-chunk
    3D DMAs split across the two hardware-DGE queues (Sync carries w,
    Scalar carries x) while output stores ride Sync in steady state.
  - Ramp: 20 dummy warm-up matmuls un-throttle the HAM clock gate while
    the first DMAs land; the first n-tile runs k-outer over mj-groups
    of 4 (half-m x loads, half-width w loads) so matmuls chase the DMA
    arrival order; the very last psum bank is drained as two half-width
    accumulation groups to shrink the tail.
"""

import os
import sys

import numpy as np

# Toolchain locations (normally already on sys.path via PYTHONPATH; be
# robust when invoked from a fresh directory/environment).
for _p in (
    "/root/.axon_site",
    "/root/.axon_site/_ro/trn_rl_repo",
    "/root/.axon_site/_ro/pypackages",
    "/opt/trn_rl_repo",
):
    if os.path.isdir(_p) and _p not in sys.path:
        sys.path.append(_p)

import ml_dtypes  # noqa: E402

BF16 = ml_dtypes.bfloat16
FP8 = ml_dtypes.float8_e4m3  # IEEE-style e4m3: matches TRN FP8_EXP4 for |v|<=240

M, K, N = 8192, 4096, 4096
N_CORES = 8
P = 128
N_TILE = 512

NFP8 = 16  # k-chunks (of 128) contracted in fp8 DoubleRow; must be even
NBF = K // P - NFP8  # k-chunks contracted in bf16
KF8 = NFP8 * P  # leading k rows in fp8


def build_nc(mc: int = M // N_CORES, n: int = N):
    """Per-core Bass program (SPMD: same program, different data)."""
    from concourse import bacc, mybir, tile

    DR = mybir.MatmulPerfMode.DoubleRow
    npair = NFP8 // 2
    mj_cnt = mc // P
    np_cnt = n // (2 * N_TILE)  # n-tile PAIRS of 1024 columns

    nc = bacc.Bacc("TRN2", target_bir_lowering=False)

    x8T = nc.dram_tensor("x8T", [KF8, mc], mybir.dt.float8e4, kind="ExternalInput")
    xbT = nc.dram_tensor("xbT", [K - KF8, mc], mybir.dt.bfloat16, kind="ExternalInput")
    w8T = nc.dram_tensor("w8T", [KF8, n], mybir.dt.float8e4, kind="ExternalInput")
    wbT = nc.dram_tensor("wbT", [K - KF8, n], mybir.dt.bfloat16, kind="ExternalInput")
    out = nc.dram_tensor("out", [mc, n], mybir.dt.float32, kind="ExternalOutput")

    x8_ap = x8T[:].rearrange("(ko p) m -> p ko m", p=P)
    xb_ap = xbT[:].rearrange("(ko p) m -> p ko m", p=P)
    w8_ap = w8T[:].rearrange("(ko p) n -> p ko n", p=P)
    wb_ap = wbT[:].rearrange("(ko p) n -> p ko n", p=P)
    out_ap = out[:].rearrange("(t p) n -> t p n", p=P)

    H = N_TILE  # 512

    with tile.TileContext(nc) as tc:
        with (
            tc.tile_pool(name="xres", bufs=1) as xpool,
            tc.tile_pool(name="warmp", bufs=1) as warmpool,
            tc.tile_pool(name="w", bufs=2) as wpool,
            tc.tile_pool(name="o", bufs=6) as opool,
            tc.tile_pool(name="ps", bufs=8, space="PSUM") as pspool,
        ):
            # HAM warm-up: dummy matmuls on a zeroed tile fill the dead
            # PE time while the prologue + first DMAs run, so the real
            # matmul stream starts at 2.4 GHz instead of 1.2.
            warm = warmpool.tile([P, N_TILE], mybir.dt.bfloat16)
            nc.vector.memset(warm[:], 0)
            warm_ps = pspool.tile([P, N_TILE], mybir.dt.float32, tag="ps")
            for _ in range(4):
                nc.tensor.matmul(warm_ps[:], warm[:, :P], warm[:], start=True, stop=True)

            x8_res = xpool.tile([P, NFP8, mc], mybir.dt.float8e4)
            xb_res = xpool.tile([P, NBF, mc], mybir.dt.bfloat16)

            def mm_steps(ps_list, w8_t, wb_t, mjs, nsl_w_list, psl=None):
                """Accumulate 25 chunk-steps into the psum tiles: one MM
                per (mj, n-slice) per step, fp8 DoubleRow pairs first
                (matching the ramp's DMA arrival order)."""
                for j in range(npair):
                    for i, mj in enumerate(mjs):
                        msl = slice(mj * P, (mj + 1) * P)
                        for t, ps in enumerate(ps_list[i]):
                            nc.tensor.matmul(
                                ps if psl is None else ps[:, psl],
                                x8_res[:, 2 * j : 2 * j + 2, msl],
                                w8_t[:, 2 * j : 2 * j + 2, nsl_w_list[t]],
                                start=(j == 0),
                                stop=False,
                                perf_mode=DR,
                            )
                for ko in range(NBF):
                    for i, mj in enumerate(mjs):
                        msl = slice(mj * P, (mj + 1) * P)
                        for t, ps in enumerate(ps_list[i]):
                            nc.tensor.matmul(
                                ps if psl is None else ps[:, psl],
                                xb_res[:, ko, msl],
                                wb_t[:, ko, nsl_w_list[t]],
                                start=False,
                                stop=(ko == NBF - 1),
                            )

            def evict(ps_ap, mj, n0, width=N_TILE):
                o_t = opool.tile([P, N_TILE], mybir.dt.float32, name="o_t")
                with tc.high_priority():
                    nc.vector.tensor_copy(out=o_t[:, :width], in_=ps_ap)
                nc.sync.dma_start(out_ap[mj, :, n0 : n0 + width], o_t[:, :width])

            # ---- n-pairs 0..3. Pair 0 is the ramp: its loads are split
            # into halves across the two hardware-DGE queues (x on Scalar,
            # w on Sync) in consumption order, and its compute runs nt0
            # k-outer (two mj-groups of 4) then nt1 mj-outer, chasing the
            # DMA arrivals. Pairs 1..3 use batched w loads (one DMA per
            # dtype, prefetched a full pair ahead) and paired mj-outer.
            for g in range(np_cnt):
                w8_t = wpool.tile([P, NFP8, 2 * N_TILE], mybir.dt.float8e4, name="w8_t")
                wb_t = wpool.tile([P, NBF, 2 * N_TILE], mybir.dt.bfloat16, name="wb_t")
                n0 = g * 2 * N_TILE
                nsl = slice(n0, n0 + 2 * N_TILE)
                if g == 0:
                    # Ramp loads: sub-batched DMAs in consumption order,
                    # x on Scalar / w on Sync (the two hardware-DGE
                    # queues; each queue is a FIFO pipe drawing from one
                    # shared ~370GB/s pool, so per-queue order must match
                    # the matmul consumption schedule).
                    # First two fp8 pairs are per-pair DMAs so the very
                    # first matmuls start ~9us instead of waiting for a
                    # whole block (fine granularity costs ~1.4us of queue
                    # time per pair, so the rest ride batches).
                    tb = [0, NBF // 3, (2 * NBF) // 3, NBF]
                    for j in range(2):
                        ksl = slice(2 * j, 2 * j + 2)
                        nc.scalar.dma_start(x8_res[:, ksl, :H], x8_ap[:, ksl, :H])
                        nc.sync.dma_start(w8_t[:, ksl, :H], w8_ap[:, ksl, :H])
                    for lo in range(4, NFP8, 6):
                        ksl = slice(lo, min(lo + 6, NFP8))
                        nc.scalar.dma_start(x8_res[:, ksl, :H], x8_ap[:, ksl, :H])
                        nc.sync.dma_start(w8_t[:, ksl, :H], w8_ap[:, ksl, :H])
                    for b in range(3):
                        ksl = slice(tb[b], tb[b + 1])
                        nc.scalar.dma_start(xb_res[:, ksl, :H], xb_ap[:, ksl, :H])
                        nc.sync.dma_start(wb_t[:, ksl, :H], wb_ap[:, ksl, :H])
                    nc.scalar.dma_start(x8_res[:, :, H:], x8_ap[:, :, H:])
                    nc.sync.dma_start(w8_t[:, :, H:], w8_ap[:, :, H : 2 * H])
                    for b in range(3):
                        ksl = slice(tb[b], tb[b + 1])
                        nc.scalar.dma_start(xb_res[:, ksl, H:], xb_ap[:, ksl, H:])
                        nc.sync.dma_start(wb_t[:, ksl, H:], wb_ap[:, ksl, H : 2 * H])

                    # nt0 (cols 0:512): k-outer over two mj-groups of 4
                    for grp in range(2):
                        pss = [
                            pspool.tile(
                                [P, N_TILE],
                                mybir.dt.float32,
                                name=f"ps0_{grp}_{i}",
                                tag="ps",
                            )
                            for i in range(4)
                        ]
                        mjs = [grp * 4 + i for i in range(4)]
                        mm_steps(
                            [[pss[i][:]] for i in range(4)],
                            w8_t,
                            wb_t,
                            mjs,
                            [slice(0, N_TILE)],
                        )
                        for i in range(4):
                            evict(pss[i][:], mjs[i], 0)

                    # nt1 (cols 512:1024): mj-outer, x resident
                    for mj in range(mj_cnt):
                        ps1 = pspool.tile(
                            [P, N_TILE], mybir.dt.float32, name=f"ps1_{mj}", tag="ps"
                        )
                        mm_steps(
                            [[ps1[:]]], w8_t, wb_t, [mj], [slice(N_TILE, 2 * N_TILE)]
                        )
                        evict(ps1[:], mj, N_TILE)
                    continue
                nc.sync.dma_start(w8_t[:, :, :], w8_ap[:, :, nsl])
                nc.sync.dma_start(wb_t[:, :, :], wb_ap[:, :, nsl])
                for mj in range(mj_cnt):
                    last = g == np_cnt - 1 and mj == mj_cnt - 1
                    ps_a = pspool.tile(
                        [P, N_TILE], mybir.dt.float32, name=f"psa_{g}_{mj}", tag="ps"
                    )
                    if not last:
                        ps_b = pspool.tile(
                            [P, N_TILE], mybir.dt.float32, name=f"psb_{g}_{mj}", tag="ps"
                        )
                        mm_steps(
                            [[ps_a[:], ps_b[:]]],
                            w8_t,
                            wb_t,
                            [mj],
                            [slice(0, N_TILE), slice(N_TILE, 2 * N_TILE)],
                        )
                        evict(ps_a[:], mj, n0)
                        evict(ps_b[:], mj, n0 + N_TILE)
                    else:
                        # Kernel-tail drain: nt0 normally, then nt1 as two
                        # sequential half-width groups so the first half's
                        # copy + store complete under the second half's
                        # matmuls.
                        mm_steps([[ps_a[:]]], w8_t, wb_t, [mj], [slice(0, N_TILE)])
                        evict(ps_a[:], mj, n0)
                        o_t = opool.tile([P, N_TILE], mybir.dt.float32)
                        h = N_TILE // 2
                        for half in range(2):
                            ps_b = pspool.tile(
                                [P, N_TILE],
                                mybir.dt.float32,
                                name=f"psbl_{g}_{mj}_{half}",
                                tag="ps",
                            )
                            hsl = slice(N_TILE + half * h, N_TILE + (half + 1) * h)
                            psl = slice(half * h, (half + 1) * h)
                            mm_steps(
                                [[ps_b]], w8_t, wb_t, [mj], [hsl], psl=psl
                            )
                            nc.vector.tensor_copy(out=o_t[:, psl], in_=ps_b[:, psl])
                            nc.sync.dma_start(
                                out_ap[
                                    mj, :, n0 + N_TILE + half * h : n0 + N_TILE + (half + 1) * h
                                ],
                                o_t[:, psl],
                            )

    return nc


_CACHE: dict = {}


def _get_finalized_nc():
    nc = _CACHE.get("nc")
    if nc is None:
        nc = build_nc()
        nc.finalize()
        _CACHE["nc"] = nc
    return nc


def _host_prep(x: np.ndarray, weight: np.ndarray):
    """Pre-sign weights, split-cast x, K-major transposes.

    Returns (x8_global [8*KF8, mc] fp8, xb_global [8*(K-KF8), mc] bf16,
             w8T [KF8, N] fp8, wbT [K-KF8, N] bf16)."""
    mc = M // N_CORES
    wb = np.sign(weight).astype(np.float32, copy=False)
    wT8 = np.ascontiguousarray(wb[:, :KF8].astype(FP8).view(np.uint8).T).view(FP8)
    wTb = (
        np.ascontiguousarray(wb[:, KF8:].astype(BF16).view(np.uint16).T).view(BF16)
    )
    # x: per-core K-major shards (transpose via integer views: ml_dtypes
    # object paths are slow for strided copies).
    x8 = np.ascontiguousarray(
        x[:, :KF8].astype(FP8).view(np.uint8).reshape(N_CORES, mc, KF8).transpose(0, 2, 1)
    )
    x8_global = x8.reshape(N_CORES * KF8, mc).view(FP8)
    xb = np.ascontiguousarray(
        x[:, KF8:].astype(BF16).view(np.uint16).reshape(N_CORES, mc, K - KF8).transpose(0, 2, 1)
    )
    xb_global = xb.reshape(N_CORES * (K - KF8), mc).view(BF16)
    return x8_global, xb_global, wT8, wTb


def make_in_maps(x: np.ndarray, weight: np.ndarray):
    x8_global, xb_global, wT8, wTb = _host_prep(x, weight)
    kb = K - KF8
    return [
        {
            "x8T": x8_global[c * KF8 : (c + 1) * KF8],
            "xbT": xb_global[c * kb : (c + 1) * kb],
            "w8T": wT8,
            "wbT": wTb,
        }
        for c in range(N_CORES)
    ]


def kernel(x: np.ndarray, weight: np.ndarray) -> np.ndarray:
    x = np.asarray(x)
    weight = np.asarray(weight)
    assert x.shape == (M, K) and weight.shape == (N, K)

    nc = _get_finalized_nc()
    from concourse.bass_utils import run_bass_kernel_spmd

    in_maps = make_in_maps(x, weight)
    try:
        res = run_bass_kernel_spmd(nc, in_maps, core_ids=list(range(N_CORES)))
    except Exception:
        # Transient device hiccups (e.g. NRT_EXEC_UNIT_UNRECOVERABLE) have
        # been observed once across many runs; one retry clears them.
        res = run_bass_kernel_spmd(nc, in_maps, core_ids=list(range(N_CORES)))
    out = np.concatenate([res.results[c]["out"] for c in range(N_CORES)], axis=0)
    return np.ascontiguousarray(out.astype(np.float32, copy=False))
